# revision 22
# baseline (speedup 1.0000x reference)
"""Trainium2 Bass kernel for nn_NatureCNN — v3: interleaved conv/LSTM.

Key structural ideas vs the baseline:
  * Frames are re-ordered time-major per core: block j of a stack holds
    times 8j..8j+8 of all 4 local segments, so the LSTM can start after
    one conv block and the whole conv trunk hides inside the LSTM's
    latency-bound 65-step chain (also keeping PE's p-state high).
  * conv1 streams 3.7x fewer PE columns: the horizontal kernel phase dx
    is folded into the contraction (K=96, host-duplicated input layout)
    and output rows are computed in 2 parity groups of 7 rows, then
    duplicated into the 4-group interleaved layout conv2 wants by the
    (otherwise idle) Act/Pool engines, fused with bias+relu.
  * lin2 is folded into the LSTM input precompute (xw): feat enters the
    LSTMs only via x@w_ih, so  dout @ Wb = relu1 @ (lin2^T Wb)  — lin2
    disappears and xw depends only on lin1's output.
  * LSTM per step: the xw gate matmul has no h dependency, so it is
    emitted one step early into the next step's PSUM bank (start=True)
    and runs in PE idle time; gates are ordered (f,i,g,o) so tanh is
    split 300+100 and the cell DVE chain starts earlier; cell-state
    scaling and the h writeback copy run on the Pool engine.
  * Weight/data DMAs are ordered so conv block 0 starts ~3us in.
"""

import numpy as np
import ml_dtypes

import concourse.bass as bass
import concourse.mybir as mybir
import concourse.tile as tile
from concourse.bass_utils import run_bass_kernel_spmd

dt = mybir.dt
AF = mybir.ActivationFunctionType
ALU = mybir.AluOpType

N_CORES = 8
S, H = 32, 100
B_LOC = 4                          # segments per core
FPC = 2 * B_LOC * S                # 256 frames per core
FB = 32                            # frames per conv block
G4 = 4 * H                         # 400
DATA_DIM, RIN, OUT = 1500, 1536, 130
NSTEP = 2 * S + 1                  # 65 layer-skewed steps

bf = ml_dtypes.bfloat16

MAX_WAITS = 1
_ctr = [0]


def _fix_sync_waits(nc):
    """This env's walrus rejects >1 sem-wait per instruction; hoist extras
    onto same-engine NoOps inserted immediately before, preserving order."""
    for f in nc.m.functions:
        for bb in f.blocks:
            new_list = []
            for ins in bb.instructions:
                si = ins.sync_info
                if si is not None and si.on_wait and len(si.on_wait) > MAX_WAITS:
                    waits = list(si.on_wait)
                    for i in range(0, len(waits) - MAX_WAITS, MAX_WAITS):
                        _ctr[0] += 1
                        nop = mybir.InstNoOp(name=f"waitfix-{_ctr[0]}", ins=[], outs=[])
                        nop.engine = ins.engine
                        nop.sync_info = mybir.SyncInfo(
                            on_wait=waits[i : i + MAX_WAITS], on_update=[])
                        new_list.append(nop)
                        nc.register_instruction(nop, overwrite=True)
                    si.on_wait = waits[len(waits) - MAX_WAITS :]
                    ins.sync_info = si
                new_list.append(ins)
            bb.instructions = new_list


# --------------------------------------------------------------------------
# host-side weight/layout prep
# --------------------------------------------------------------------------

def _gate_perm():
    # torch gate order (i, f, g, o) -> ours (f, i, g, o)
    return np.concatenate([np.arange(100, 200), np.arange(0, 100),
                           np.arange(200, 300), np.arange(300, 400)])


def _feat_ch_order():
    """our featT row R -> original feat channel index."""
    orig = np.zeros(RIN, np.int64)
    for R in range(1024):
        c, rem = divmod(R, 128)
        g, oc3 = divmod(rem, 64)
        orig[R] = oc3 * 16 + (2 * g + c // 4) * 4 + (c % 4)
    orig[1024:] = np.arange(1024, RIN)
    return orig


def _frame_perm():
    # new-order index (j,s,tau) -> old core-local index 32s + 8j + tau
    jj, ss, tt = np.meshgrid(np.arange(4), np.arange(4), np.arange(8),
                             indexing='ij')
    return (32 * ss + 8 * jj + tt).reshape(-1)


def _prep_weights(inp):
    w = {}
    gp = _gate_perm()
    ch = _feat_ch_order()

    # conv1, dx folded into contraction: W[(ic,ry,rx,dx), dy, oc]
    c1 = inp['conv1_w'].reshape(32, 3, 2, 4, 2, 4)   # oc, ic, dy, ry, dx, rx
    w['w1p'] = np.ascontiguousarray(
        c1.transpose(1, 3, 5, 4, 2, 0).reshape(96, 2 * 32)).astype(bf)
    w['cb1c'] = np.tile(inp['conv1_b'], 2).reshape(64, 1).astype(np.float32)
    w['w2'] = np.ascontiguousarray(
        inp['conv2_w'].transpose(3, 2, 1, 0).reshape(4, 128, 64)
        .transpose(1, 0, 2).reshape(128, 4 * 64)).astype(bf)
    w['cb2'] = inp['conv2_b'].reshape(64, 1).astype(np.float32)
    w['w3'] = np.ascontiguousarray(
        inp['conv3_w'].transpose(2, 3, 1, 0).reshape(9, 64, 64)
        .transpose(1, 0, 2).reshape(64, 9 * 64)).astype(bf)
    w['cb3'] = np.tile(inp['conv3_b'], 2).reshape(128, 1).astype(np.float32)

    l1 = np.zeros((RIN, 1024), np.float32)
    l1[:DATA_DIM] = inp['lin1_w'].T
    w['lin1wT'] = np.ascontiguousarray(
        l1.reshape(12, 128, 1024).transpose(1, 0, 2).reshape(128, 12 * 1024)).astype(bf)
    w['b1col'] = inp['lin1_b'].reshape(8, 128).T.copy().astype(np.float32)

    def lstm_prep(pfx):
        wih0 = inp[f'{pfx}_wih0'][gp].copy()
        wih0[200:300] *= 2.0
        wih0T = wih0.T[ch].copy()                     # [1536, 400]
        wa = wih0T[:1024]                             # c3 part
        wb = wih0T[1024:RIN]                          # dout part [512-pad.., 400]
        # fold lin2: xw_dout = relu1 @ (lin2w^T @ wb); bias b2 @ wb into b0
        whb = inp['lin2_w'].T @ wb                    # [1024, 400]
        b0 = (inp[f'{pfx}_bih0'] + inp[f'{pfx}_bhh0'])[gp].copy()
        b0[200:300] *= 2.0
        b0 = b0 + inp['lin2_b'] @ wb
        whh0 = inp[f'{pfx}_whh0'][gp].copy()
        whh0[200:300] *= 2.0
        wih1 = inp[f'{pfx}_wih1'][gp].copy()
        wih1[200:300] *= 2.0
        whh1 = inp[f'{pfx}_whh1'][gp].copy()
        whh1[200:300] *= 2.0
        b1 = (inp[f'{pfx}_bih1'] + inp[f'{pfx}_bhh1'])[gp].copy()
        b1[200:300] *= 2.0
        whh1ext = np.concatenate([whh1.T * 0.5, b1[None, :]], 0)
        whh0x = np.concatenate([whh0.T * 0.5, np.zeros((1, G4), np.float32)], 0)
        return wa, whb, b0, whh0x, wih1.T * 0.5, whh1ext

    a = [lstm_prep('rnns'), lstm_prep('rnn')]
    w['waT'] = np.ascontiguousarray(
        np.stack([x[0] for x in a]).reshape(2, 8, 128, G4)
        .transpose(2, 0, 1, 3).reshape(128, 16 * G4)).astype(bf)
    w['whbT'] = np.ascontiguousarray(
        np.stack([x[1] for x in a]).reshape(2, 8, 128, G4)
        .transpose(2, 0, 1, 3).reshape(128, 16 * G4)).astype(bf)
    w['b0row'] = np.stack([x[2] for x in a]).reshape(1, 2 * G4).astype(bf)
    w['whh0T'] = np.stack([x[3] for x in a]).astype(bf)
    w['wih1T'] = np.ascontiguousarray(
        np.stack([x[4] for x in a]).transpose(1, 0, 2).reshape(100, 2 * G4)).astype(bf)
    w['whh1T'] = np.ascontiguousarray(
        np.stack([x[5] for x in a]).transpose(1, 0, 2).reshape(101, 2 * G4)).astype(bf)

    w['fc1wT'] = np.ascontiguousarray(
        np.stack([inp['fc1_w'][:, :100].T * 0.5, inp['fc1_w'][:, 100:].T * 0.5])
        .transpose(1, 0, 2).reshape(100, 2 * 512)).astype(bf)
    w['fc1bcol'] = inp['fc1_b'].reshape(4, 128).T.copy().astype(np.float32)
    w['fc2wT'] = np.ascontiguousarray(
        inp['fc2_w'].T.reshape(4, 128, OUT).transpose(1, 0, 2)
        .reshape(128, 4 * OUT)).astype(bf)
    w['fc2brow'] = inp['fc2_b'].reshape(1, OUT).astype(bf)

    i4 = np.zeros((128, 4), np.float32)
    for k in range(4):
        i4[32 * k : 32 * k + 4] = np.eye(4)
    w['i4rep'] = i4.astype(bf)
    w['ident36'] = np.eye(36).astype(bf)
    w['ones128'] = np.ones((1, 128), np.float32).astype(bf)
    w['ones4'] = np.ones((1, 4), np.float32).astype(bf)
    hti = np.zeros((105, 36), np.float32)
    hti[100, :] = 1.0
    hti[101:105, 0:4] = np.eye(4)
    w['hT_init'] = hti.astype(bf)
    w['zeros128'] = np.zeros((128, 512), np.float32)
    w['zerosbf'] = np.zeros((36, 100), np.float32).astype(bf)
    return w


def _prep_core_inputs(inp, w, k):
    fp = _frame_perm()
    idx = np.concatenate([128 * k + fp, 1024 + 128 * k + fp])
    obs = np.asarray(inp['observations'], np.float32)[idx]
    # [(ic,ry,rx,dx)=96, f, by=16, ox=15], laid out per conv block
    o2 = (obs.reshape(FPC, 3, 16, 4, 16, 4)
          .transpose(1, 3, 5, 0, 2, 4))                 # ic, ry, rx, f, by, bx
    ob2 = np.stack([o2[..., 0:15], o2[..., 1:16]], axis=3)  # ic,ry,rx,dx,f,by,ox
    m = {'obsT2': np.ascontiguousarray(
            ob2.reshape(96, FPC, 240).reshape(96, 8, FB * 240)).astype(bf)}
    data = np.asarray(inp['data'], np.float32)[idx]
    dT = np.zeros((RIN, FPC), np.float32)
    dT[:DATA_DIM] = data.T
    m['dataT'] = np.ascontiguousarray(
        dT.reshape(12, 128, FPC).transpose(1, 0, 2).reshape(128, 12 * FPC)).astype(bf)
    m.update(w)
    return m


# --------------------------------------------------------------------------
# kernel IR
# --------------------------------------------------------------------------

def _build_nc(debug=False):
    nc = bass.Bass("TRN2", target_bir_lowering=False, debug=False,
                   num_devices=N_CORES)

    D = {}
    def inp(name, shape, d):
        D[name] = nc.dram_tensor(name, shape, d, kind="ExternalInput")

    inp('obsT2', [96, 8, FB * 240], dt.bfloat16)
    inp('dataT', [128, 12 * FPC], dt.bfloat16)
    inp('w1p', [96, 64], dt.bfloat16)
    inp('cb1c', [64, 1], dt.float32)
    inp('w2', [128, 4 * 64], dt.bfloat16)
    inp('cb2', [64, 1], dt.float32)
    inp('w3', [64, 9 * 64], dt.bfloat16)
    inp('cb3', [128, 1], dt.float32)
    inp('lin1wT', [128, 12 * 1024], dt.bfloat16)
    inp('b1col', [128, 8], dt.float32)
    inp('waT', [128, 16 * G4], dt.bfloat16)
    inp('whbT', [128, 16 * G4], dt.bfloat16)
    inp('b0row', [1, 2 * G4], dt.bfloat16)
    inp('whh0T', [2, 101, G4], dt.bfloat16)
    inp('wih1T', [100, 2 * G4], dt.bfloat16)
    inp('whh1T', [101, 2 * G4], dt.bfloat16)
    inp('fc1wT', [100, 2 * 512], dt.bfloat16)
    inp('fc1bcol', [128, 4], dt.float32)
    inp('fc2wT', [128, 4 * OUT], dt.bfloat16)
    inp('fc2brow', [1, OUT], dt.bfloat16)
    inp('i4rep', [128, 4], dt.bfloat16)
    inp('ident36', [36, 36], dt.bfloat16)
    inp('ones128', [1, 128], dt.bfloat16)
    inp('ones4', [1, 4], dt.bfloat16)
    inp('hT_init', [105, 36], dt.bfloat16)
    inp('zeros128', [128, 512], dt.float32)
    inp('zerosbf', [36, 100], dt.bfloat16)

    out_d = nc.dram_tensor('out', [B_LOC, OUT], dt.float32, kind="ExternalOutput")
    xw_scr = nc.dram_tensor('xw_scr', [2, 4, 32, G4], dt.bfloat16)
    dbg = {}
    if debug:
        dbg['d_c3'] = nc.dram_tensor('d_c3', [2, 128, 128 * 8], dt.bfloat16, kind="ExternalOutput")
        dbg['d_relu1'] = nc.dram_tensor('d_relu1', [128, 8 * FPC], dt.bfloat16, kind="ExternalOutput")
        dbg['d_xw'] = nc.dram_tensor('d_xw', [2, 128, 8 * G4], dt.bfloat16, kind="ExternalOutput")
        dbg['d_hT'] = nc.dram_tensor('d_hT', [101, 36], dt.bfloat16, kind="ExternalOutput")
        dbg['d_q'] = nc.dram_tensor('d_q', [100, 4], dt.bfloat16, kind="ExternalOutput")
        dbg['d_R'] = nc.dram_tensor('d_R', [128, FB * 84], dt.bfloat16, kind="ExternalOutput")

    with tile.TileContext(nc) as tc:
        with (
            tc.tile_pool(name="const", bufs=1) as cpool,
            tc.tile_pool(name="acts", bufs=1) as apool,
            tc.tile_pool(name="conv", bufs=2) as vpool,
            tc.tile_pool(name="lstm", bufs=2) as lpool,
        ):
            def ld(name, shape, d, tag=None):
                t = cpool.tile(shape, d, tag=tag or name, name=tag or name)
                nc.sync.dma_start(t[:], D[name][:])
                return t

            def ld_stack(name, p, a, wdt, inner, tag=None):
                # DRAM [a, p, inner] -> SBUF [p, a*inner]
                t = cpool.tile([p, a * inner], wdt, tag=tag or name,
                               name=tag or name)
                nc.sync.dma_start(t[:].rearrange("p (a g) -> p a g", a=a),
                                  D[name][:].rearrange("a p g -> p a g"))
                return t

            def ld_split(name, shape, d, parts, pool=None, tag=None):
                # big tensors split across several dma_starts so the
                # transfers round-robin onto parallel DMA rings
                t = (pool or cpool).tile(shape, d, tag=tag or name,
                                         name=tag or name)
                n = shape[1]
                step = -(-n // parts)
                for a in range(0, n, step):
                    b = min(a + step, n)
                    nc.sync.dma_start(t[:, a:b], D[name][:, a:b])
                return t

            # --- DMA priority order: obsT2-A0, conv wts, dataT, obsT2-A1,
            #     lin1w, xw wts, lstm wts+state, fc wts ---
            ob00 = vpool.tile([96, FB * 240], dt.bfloat16, tag="ob2",
                              name="ob2", bufs=2)
            nc.sync.dma_start(ob00[:, 0:FB * 120], D['obsT2'][:, 0, 0:FB * 120])
            nc.sync.dma_start(ob00[:, FB * 120:], D['obsT2'][:, 0, FB * 120:])
            w1_s = ld('w1p', [96, 64], dt.bfloat16)
            cb1_s = ld('cb1c', [64, 1], dt.float32)
            w2_s = ld('w2', [128, 4 * 64], dt.bfloat16)
            cb2_s = ld('cb2', [64, 1], dt.float32)
            w3_s = ld('w3', [64, 9 * 64], dt.bfloat16)
            cb3_s = ld('cb3', [128, 1], dt.float32)
            zeros_s = ld('zeros128', [128, 512], dt.float32)

            # LSTM weights/state: tiny but step-0-critical -> DMA first
            wih1_s = ld('wih1T', [100, 2 * G4], dt.bfloat16)
            whh1_s = ld('whh1T', [101, 2 * G4], dt.bfloat16)
            id36_s = ld('ident36', [36, 36], dt.bfloat16)
            i4_s = ld('i4rep', [128, 4], dt.bfloat16)
            hT = []
            for i in range(2):
                hTt = lpool.tile([105, 36], dt.bfloat16, tag=f"hT{i}",
                                 name=f"hT{i}", bufs=1)
                hT.append(hTt)
                nc.sync.dma_start(hTt[:], D['hT_init'][:])
            # W0X[lo][par]: [105, G4] fused l0 moving operand: rows 0:100
            # whh0[lo], row 100 zero, rows 101:105 per-step xw (DMA-staged)
            w0x = [[lpool.tile([105, G4], dt.bfloat16, name=f"w0x{lo}{p}",
                               tag=f"w0x{lo}{p}", bufs=1)
                    for p in range(2)] for lo in range(2)]
            for lo in range(2):
                for p in range(2):
                    nc.sync.dma_start(w0x[lo][p][0:101, :], D['whh0T'][lo])
            h2t = lpool.tile([36, 100], dt.bfloat16, tag="h2t", bufs=1)
            nc.sync.dma_start(h2t[:], D['zerosbf'][:])
            sC = lpool.tile([36, 100], dt.float32, tag="sC", bufs=1)
            nc.sync.dma_start(sC[:], D['zeros128'][0:36, 0:100])
            qsave = lpool.tile([100, 4], dt.bfloat16, tag="qsave", bufs=1)
            wih1v = wih1_s[:].rearrange("p (a g) -> p a g", a=2)
            whh1v = whh1_s[:].rearrange("p (a g) -> p a g", a=2)
            b0_s = ld('b0row', [1, 2 * G4], dt.bfloat16)
            ones128_s = ld('ones128', [1, 128], dt.bfloat16)

            b1c_s = ld('b1col', [128, 8], dt.float32)
            dataT_s = ld_split('dataT', [128, 12 * FPC], dt.bfloat16, 2,
                               pool=apool)
            ob01 = vpool.tile([96, FB * 240], dt.bfloat16, tag="ob2",
                              name="ob2", bufs=2)
            nc.sync.dma_start(ob01[:, 0:FB * 120], D['obsT2'][:, 1, 0:FB * 120])
            nc.sync.dma_start(ob01[:, FB * 120:], D['obsT2'][:, 1, FB * 120:])
            lin1w_s = ld_split('lin1wT', [128, 12 * 1024], dt.bfloat16, 4)

            c3t = [apool.tile([128, 128 * 8], dt.bfloat16, tag=f"c3{x}",
                              name=f"c3{x}") for x in "AB"]
            relu1_s = apool.tile([128, 8 * FPC], dt.bfloat16, tag="relu1")
            xwpad = [apool.tile([128, 8 * G4], dt.bfloat16, tag=f"xwpad{l}",
                                name=f"xwpad{l}") for l in range(2)]

            w1v = w1_s[:].rearrange("p (a b) -> p a b", a=2)
            w2v = w2_s[:].rearrange("p (a b) -> p a b", a=4)
            w3v = w3_s[:].rearrange("p (a b) -> p a b", a=9)

            with (
                tc.tile_pool(name="cps", bufs=3, space="PSUM") as psc,
                tc.tile_pool(name="psx", bufs=1, space="PSUM") as psxp,
                tc.tile_pool(name="psl", bufs=2, space="PSUM") as psl,
            ):
                # ================= work generators =================
                def conv_gen(stk, blk, ob=None):
                    gblk = 4 * stk + blk
                    if ob is None:
                        ob = vpool.tile([96, FB * 240], dt.bfloat16, tag="ob2",
                                        name="ob2", bufs=2)
                        nc.sync.dma_start(ob[:, 0:FB * 120],
                                          D['obsT2'][:, gblk, 0:FB * 120])
                        nc.sync.dma_start(ob[:, FB * 120:],
                                          D['obsT2'][:, gblk, FB * 120:])
                    v2 = ob[:].rearrange("p (f by ox) -> p f by ox", f=FB, by=16)
                    rt = vpool.tile([128, FB * 84], dt.bfloat16, tag="rt",
                                    name="rt", bufs=2)
                    rv4 = rt[:].rearrange("p (f m x) -> p f m x", f=FB, m=6)
                    for fg in range(8):
                        fa = 4 * fg
                        P2 = psc.tile([128, 512], dt.float32, tag="cps", name="cps")
                        for q in (0, 1):
                            for dy in (0, 1):
                                mov = v2[:, fa:fa + 4, q + dy:q + dy + 13:2, 0:14]
                                nc.tensor.matmul(
                                    P2[32 * q:32 * q + 32, 0:392],
                                    w1v[:, dy, :], mov,
                                    start=(dy == 0), stop=(dy == 1),
                                    tile_position=(0, 32 * q))
                        pv = P2[0:64, 0:392].rearrange(
                            "p (f m x) -> p f m x", f=4, m=7)
                        # relu+bias fused with the parity->4-group duplication
                        nc.scalar.activation(rv4[0:64, fa:fa + 4, :, :],
                                             pv[:, :, 0:6, :], AF.Relu,
                                             bias=cb1_s[:], scale=1.0)
                        nc.vector.scalar_tensor_tensor(
                            rv4[64:128, fa:fa + 4, :, :], pv[:, :, 1:7, :],
                            cb1_s[:], zeros_s[0:64, 0:336], ALU.add, ALU.max)
                        yield
                    rv = rt[:].rearrange("p (f m x) -> p f m x", m=6, x=14)
                    c2 = vpool.tile([64, FB * 36], dt.bfloat16, tag="c2",
                                    name="c2", bufs=2)
                    for (a, b) in [(0, 12), (12, 24), (24, 32)]:
                        ncols = (b - a) * 36
                        ps2 = psc.tile([128, 512], dt.float32, tag="cps", name="cps")
                        for kx in range(4):
                            mov = rv[:, a:b, :, kx:kx + 11:2]
                            nc.tensor.matmul(ps2[0:64, :ncols], w2v[:, kx, :], mov,
                                             start=(kx == 0), stop=(kx == 3))
                        nc.scalar.activation(c2[:, a * 36:b * 36], ps2[0:64, :ncols],
                                             AF.Relu, bias=cb2_s[:], scale=1.0)
                        yield
                    c2v = c2[:].rearrange("p (f a b) -> p f a b", a=6, b=6)
                    ps3 = psc.tile([128, 512], dt.float32, tag="cps", name="cps")
                    for g in range(2):
                        for ki, (ky, kx) in enumerate(
                                [(y, x) for y in range(3) for x in range(3)]):
                            mov = c2v[:, :, 2 * g + ky:2 * g + ky + 2, kx:kx + 4]
                            nc.tensor.matmul(ps3[64 * g:64 * g + 64, 0:FB * 8],
                                             w3v[:, 3 * ky + kx, :], mov,
                                             start=(ki == 0), stop=(ki == 8),
                                             tile_position=(0, 64 * g))
                        yield
                    nc.scalar.activation(c3t[stk][:, blk * FB * 8:(blk + 1) * FB * 8],
                                         ps3[:, 0:FB * 8], AF.Relu,
                                         bias=cb3_s[:], scale=1.0)
                    if debug and stk == 0 and blk == 0:
                        nc.sync.dma_start(dbg['d_R'][:], rt[:])
                    yield

                def lin1_gen():
                    l1v = lin1w_s[:].rearrange("p (a g) -> p a g", a=12)
                    for m in range(8):
                        ph = psc.tile([128, 512], dt.float32, tag="cps", name="cps")
                        for kc in range(12):
                            nc.tensor.matmul(ph[:, 0:FPC],
                                             l1v[:, kc, 128 * m:128 * m + 128],
                                             dataT_s[:, FPC * kc:FPC * (kc + 1)],
                                             start=(kc == 0), stop=(kc == 11))
                        nc.scalar.activation(relu1_s[:, FPC * m:FPC * (m + 1)],
                                             ph[:, 0:FPC], AF.Relu,
                                             bias=b1c_s[:, m:m + 1], scale=1.0)
                        yield

                def xw_gen(stk, blk):
                    psx = psxp.tile([32, G4], dt.float32, tag="psx", name="psx")
                    for kc in range(8):
                        base = 8 * FB * blk + kc
                        stat = c3t[stk][:, base: base + 8 * (FB - 1) + 1: 8]
                        nc.tensor.matmul(psx[:], stat, wav[:, stk, kc, :],
                                         start=(kc == 0), stop=False)
                        if kc == 5:
                            yield
                    yield
                    for kc in range(8):
                        base = FPC * kc + 128 * stk + FB * blk
                        stat = relu1_s[:, base:base + FB]
                        nc.tensor.matmul(psx[:], stat, whbv[:, stk, kc, :],
                                         start=False, stop=False)
                        if kc == 5:
                            yield
                    nc.tensor.matmul(psx[:], ones128_s[:, 0:32], b0v[:, stk, :],
                                     start=False, stop=True)
                    xwc = lpool.tile([32, G4], dt.bfloat16, tag="xwc", name="xwc",
                                     bufs=2)
                    nc.scalar.activation(xwc[:], psx[:], AF.Copy, bias=0.0, scale=1.0)
                    nc.sync.dma_start(xw_scr[stk, blk], xwc[:])
                    # scatter rows (s,tau): r=8s+tau -> pad partition 32*(tau%4)+s,
                    # col chunk 2*blk + tau//4  (matches the i4 gather in xw_mm)
                    srcv = xw_scr[stk, blk].rearrange("(s t2 t4) g -> t2 t4 s g",
                                                      s=4, t2=2)
                    for t2 in range(2):
                        for t4 in range(4):
                            dst = xwpad[stk][32 * t4:32 * t4 + 4,
                                             (2 * blk + t2) * G4:
                                             (2 * blk + t2 + 1) * G4]
                            nc.sync.dma_start(dst, srcv[t2, t4])
                    yield

                def xw_stage(s):
                    # stage xw rows for step s into W0X[lo(s)][s%2] rows
                    # 101:105 via SBUF->SBUF DMA (engines can't address
                    # partition base 101, DMA can); runs well off-path
                    lo_, t0 = (0 if s < 32 else 1), s % 32
                    q4, tg = 32 * (t0 % 4), t0 // 4
                    nc.sync.dma_start(
                        w0x[lo_][s % 2][101:105, :],
                        xwpad[lo_][q4:q4 + 4, tg * G4:(tg + 1) * G4])

                # ================= schedule =================
                # XW weights (DMA-ordered after obsT2-A1/lin1w)
                wa_s = ld_split('waT', [128, 16 * G4], dt.bfloat16, 2)
                whb_s = ld_split('whbT', [128, 16 * G4], dt.bfloat16, 2)
                wav = wa_s[:].rearrange("p (a c g) -> p a c g", a=2, c=8)
                whbv = whb_s[:].rearrange("p (a c g) -> p a c g", a=2, c=8)
                b0v = b0_s[:].rearrange("p (a g) -> p a g", a=2)

                # preamble PE work: conv A0, conv A1, lin1 (all frames), xw A0
                work = []    # [gen, deadline_step)
                for g in conv_gen(0, 0, ob=ob00):
                    pass
                pre_a1 = conv_gen(0, 1, ob=ob01)
                for _ in range(6):
                    next(pre_a1)
                for g in lin1_gen():
                    pass
                for g in xw_gen(0, 0):
                    pass
                xw_stage(0)
                xw_stage(1)

                work.append([pre_a1, 5])
                work.append([xw_gen(0, 1), 5])
                for blk in range(2, 4):
                    work.append([conv_gen(0, blk), 8 * blk - 3])
                    work.append([xw_gen(0, blk), 8 * blk - 2])
                for blk in range(4):
                    work.append([conv_gen(1, blk), 32 + 8 * blk - 3])
                    work.append([xw_gen(1, blk), 32 + 8 * blk - 2])

                def advance(n):
                    while n > 0 and work:
                        try:
                            next(work[0][0])
                            n -= 1
                        except StopIteration:
                            work.pop(0)

                def drain_due(s):
                    while work and work[0][1] <= s:
                        try:
                            next(work[0][0])
                        except StopIteration:
                            work.pop(0)

                # ================= LSTM loop =================
                for s_ in range(NSTEP):
                    drain_due(s_)
                    l0_act = s_ <= 63
                    l1_act = 1 <= s_
                    lo = 0 if s_ < 32 else 1
                    l1i = 0 if (s_ - 1) < 32 else 1
                    hp = hT[(s_ - 1) % 2]
                    hn = hT[s_ % 2]
                    p0, p1 = (0, 4) if s_ == 0 else ((32, 36) if s_ == 64 else (0, 36))

                    G = psl.tile([36, G4], dt.float32, tag="gpsum",
                                 name="gpsum", bufs=2)
                    if l0_act:
                        nc.tensor.matmul(G[0:4, :], hp[0:105, 0:4],
                                         w0x[lo][s_ % 2][:],
                                         start=True, stop=True,
                                         tile_position=(0, 0))
                    if l1_act:
                        nc.tensor.matmul(G[32:36, :], hp[0:100, 0:4], wih1v[:, l1i, :],
                                         start=True, stop=False, tile_position=(0, 32))
                        nc.tensor.matmul(G[32:36, :], hp[0:101, 32:36], whh1v[:, l1i, :],
                                         start=False, stop=True, tile_position=(0, 32))
                    if l0_act and s_ + 2 <= 63:
                        xw_stage(s_ + 2)

                    # gate cols: f 0:100, i 100:200, g 200:300, o 300:400
                    T = lpool.tile([36, G4], dt.float32, tag="tanhT", name="tanhT")
                    nc.scalar.activation(T[p0:p1, 0:300], G[p0:p1, 0:300], AF.Tanh,
                                         bias=0.0, scale=0.5)
                    u = lpool.tile([36, 100], dt.float32, tag="ut", name="ut")
                    v = lpool.tile([36, 100], dt.float32, tag="vt", name="vt")
                    cn = lpool.tile([36, 100], dt.float32, tag="cnt", name="cnt")
                    th = lpool.tile([36, 100], dt.float32, tag="tht", name="tht")
                    nc.vector.scalar_tensor_tensor(u[p0:p1, :], T[p0:p1, 0:100], 1.0,
                                                   sC[p0:p1, :], ALU.add, ALU.mult)
                    nc.vector.scalar_tensor_tensor(v[p0:p1, :], T[p0:p1, 100:200], 1.0,
                                                   T[p0:p1, 200:300], ALU.add, ALU.mult)
                    nc.scalar.activation(T[p0:p1, 300:400], G[p0:p1, 300:400], AF.Tanh,
                                         bias=0.0, scale=0.5)
                    nc.vector.scalar_tensor_tensor(cn[p0:p1, :], v[p0:p1, :], 0.5,
                                                   u[p0:p1, :], ALU.mult, ALU.add)
                    nc.vector.tensor_scalar_mul(sC[p0:p1, :], cn[p0:p1, :], 0.5)
                    nc.scalar.activation(th[p0:p1, :], cn[p0:p1, :], AF.Tanh,
                                         bias=0.0, scale=1.0)
                    nc.vector.scalar_tensor_tensor(
                        h2t[p0:p1, :].bitcast(dt.bfloat16), T[p0:p1, 300:400], 1.0,
                        th[p0:p1, :], ALU.add, ALU.mult)
                    # deferred conv/xw slices fill the PE gap before the
                    # transpose; their relu ops land after this step's cell
                    # ops on Act/DVE so they don't delay the tanh chain
                    advance(2)
                    pst = psl.tile([100, 36], dt.bfloat16, tag="pshT", name="pshT",
                                   bufs=2)
                    nc.tensor.transpose(pst[:], h2t[:], id36_s[:])
                    nc.vector.tensor_copy(hn[0:100, :], pst[:])
                    if s_ == 32:
                        nc.scalar.copy(qsave[:], hn[0:100, 32:36])

                # drain any remaining deferred work (shouldn't happen)
                while work:
                    try:
                        next(work[0][0])
                    except StopIteration:
                        work.pop(0)

            # ========================= head =========================
            fc1w_s = ld('fc1wT', [100, 2 * 512], dt.bfloat16)
            fc1b_s = ld('fc1bcol', [128, 4], dt.float32)
            fc2w_s = ld('fc2wT', [128, 4 * OUT], dt.bfloat16)
            fc2b_s = ld('fc2brow', [1, OUT], dt.bfloat16)
            ones4_s = ld('ones4', [1, 4], dt.bfloat16)
            with tc.tile_pool(name="pshead", bufs=1, space="PSUM") as psh:
                hlast = hT[(NSTEP - 1) % 2]
                psf1 = psh.tile([128, 16], dt.float32, tag="psf1")
                fc1v = fc1w_s[:].rearrange("p (a g) -> p a g", a=2)
                for m in range(4):
                    nc.tensor.matmul(psf1[:, 4 * m:4 * m + 4],
                                     fc1v[:, 0, 128 * m:128 * m + 128],
                                     hlast[0:100, 32:36], start=True, stop=False)
                    nc.tensor.matmul(psf1[:, 4 * m:4 * m + 4],
                                     fc1v[:, 1, 128 * m:128 * m + 128],
                                     qsave[:], start=False, stop=True)
                z1 = lpool.tile([128, 16], dt.bfloat16, tag="z1t", bufs=1)
                for m in range(4):
                    nc.scalar.activation(z1[:, 4 * m:4 * m + 4],
                                         psf1[:, 4 * m:4 * m + 4], AF.Relu,
                                         bias=fc1b_s[:, m:m + 1], scale=1.0)
                psf2 = psh.tile([4, OUT], dt.float32, tag="psf2")
                fc2v = fc2w_s[:].rearrange("p (a g) -> p a g", a=4)
                for m in range(4):
                    nc.tensor.matmul(psf2[:], z1[:, 4 * m:4 * m + 4], fc2v[:, m, :],
                                     start=(m == 0), stop=False)
                nc.tensor.matmul(psf2[:], ones4_s[:], fc2b_s[:],
                                 start=False, stop=True)
                ot = lpool.tile([4, OUT], dt.float32, tag="outt", bufs=1)
                nc.scalar.copy(ot[:], psf2[:])
                nc.sync.dma_start(out_d[:], ot[:])

            if debug:
                for l in range(2):
                    nc.sync.dma_start(dbg['d_c3'][l], c3t[l][:])
                    nc.sync.dma_start(dbg['d_xw'][l], xwpad[l][:])
                nc.sync.dma_start(dbg['d_relu1'][:], relu1_s[:])
                nc.sync.dma_start(dbg['d_hT'][:], hT[(NSTEP - 1) % 2][:])
                nc.sync.dma_start(dbg['d_q'][:], qsave[:])

    _fix_sync_waits(nc)
    return nc


_NC_CACHE = {}


def _run(inputs, debug=False):
    inputs = {k: np.asarray(v) for k, v in inputs.items()}
    winp = {k: (np.asarray(v, np.float32) if np.asarray(v).ndim else v)
            for k, v in inputs.items()}
    w = _prep_weights(winp)
    key = ('nc', debug)
    if key not in _NC_CACHE:
        _NC_CACHE[key] = _build_nc(debug=debug)
    nc = _NC_CACHE[key]
    in_maps = [_prep_core_inputs(inputs, w, k) for k in range(N_CORES)]
    res = run_bass_kernel_spmd(nc, in_maps, core_ids=list(range(N_CORES)))
    return res


def kernel(**inputs):
    res = _run(inputs, debug=False)
    out = np.concatenate([res.results[k]['out'] for k in range(N_CORES)], 0)
    return out.astype(np.float32)
